# revision 16
# baseline (speedup 1.0000x reference)
"""Trainium2 Bass kernel for nn_CEABlock_Enhancement (CEA block, dual-modality).

Sharding: pure data parallel over batch B=32 across 8 NeuronCores (4 per core).
Layout: channel-major bf16 activations [128 chan, chunk, tokens]; LN affine
folded into downstream weights (host-side); attention softmax Z via ACT
accum_out; P^T / V^T for the PV matmul via PE transposes.
"""

import os
import sys

sys.path.insert(0, "/opt/trn_rl_repo")

_STOP = os.environ.get("CEA_STOP", "full")  # a|b|c|d|full (build bisect)
_SKIP = set(os.environ.get("CEA_SKIP", "").split(","))

from contextlib import ExitStack

import numpy as np
import ml_dtypes

import concourse.bass as bass  # noqa: F401
import concourse.mybir as mybir
import concourse.tile as tile
from concourse import bacc, masks
from concourse.bass_utils import run_bass_kernel_spmd

F32 = mybir.dt.float32
BF16 = mybir.dt.bfloat16
AF = mybir.ActivationFunctionType
OP = mybir.AluOpType

# Problem constants
B, ZL, SL, DIM, HEADS, HID = 32, 64, 256, 768, 12, 3072
HD = DIM // HEADS          # 64
NT = ZL + SL               # 320 tokens in x
NC3 = ZL + NT              # 384 tokens in xc
DC = DIM // 128            # 6 channel chunks
QC = 3 * DC                # 18 qkv chunks
HC = HID // 128            # 24 hidden chunks
NCORES = 8
NB = B // NCORES           # 4 batch elems per core
SCALE = DIM ** -0.5
EPS = 1e-5

_CACHED = {}


def _tok_chunks(n):
    out, t = [], 0
    while t < n:
        out.append((t, min(128, n - t)))
        t += 128
    return out


def build_nc():
    nc = bacc.Bacc("TRN2", target_bir_lowering=False, debug=False,
                   num_devices=NCORES)

    # ---------------- DRAM parameters ----------------
    x_d = nc.declare_dram_parameter("x", [NB, NT, DIM], F32, isOutput=False)
    xi_d = nc.declare_dram_parameter("xi", [NB, NT, DIM], F32, isOutput=False)

    def wparam(name, shape, dt=BF16):
        return nc.declare_dram_parameter(name, shape, dt, isOutput=False)

    wal_d = wparam("wal", [128, 2 * DC, DC, 128])
    wqkv_d = wparam("wqkv", [128, DC, QC, 128])
    wproj_d = wparam("wproj", [128, DC, DC, 128])
    wfus_d = wparam("wfus", [128, DC, DC, 128])
    wal2_d = wparam("wal2", [128, DC, DC, 128])
    wfc1_d = wparam("wfc1", [128, DC, HC, 128])
    wfc2_d = wparam("wfc2", [128, HC, DC, 128])
    bal_d = wparam("bal", [128, DC], F32)
    bqkv_d = wparam("bqkv", [128, QC], F32)
    bprow_d = wparam("bprow", [1, DC * 128])    # proj bias as one row
    bfus_d = wparam("bfus", [128, DC], F32)
    bal2_d = wparam("bal2", [128, DC], F32)
    bfc1_d = wparam("bfc1", [128, HC], F32)
    bfc2_d = wparam("bfc2", [128, DC], F32)
    g1_d = wparam("g1c", [128, DC], F32)
    bt1_d = wparam("bt1c", [128, DC])

    xo_d = nc.declare_dram_parameter("xo", [NB, NT, DIM], F32, isOutput=True)
    xio_d = nc.declare_dram_parameter("xio", [NB, NT, DIM], F32, isOutput=True)
    at_d = nc.declare_dram_parameter("attn", [NB, HEADS, NC3, NC3], BF16,
                                     isOutput=True)
    ati_d = nc.declare_dram_parameter("attni", [NB, HEADS, NC3, NC3], BF16,
                                      isOutput=True)
    al_d = nc.declare_dram_parameter("alpha", [NB, SL], F32, isOutput=True)
    um_d = nc.declare_dram_parameter("um", [NB, SL], F32, isOutput=True)
    u_d = nc.declare_dram_parameter("u", [NB, SL], F32, isOutput=True)
    ui_d = nc.declare_dram_parameter("ui", [NB, SL], F32, isOutput=True)

    MODS = [0, 1]
    xin = [x_d, xi_d]
    attn_out = [at_d, ati_d]
    out_u = [u_d, ui_d]
    xout_d = [xo_d, xio_d]

    with tile.TileContext(nc) as tc, ExitStack() as ctx:
        consts = ctx.enter_context(tc.tile_pool(name="consts", bufs=1))
        wpool = ctx.enter_context(tc.tile_pool(name="wpool", bufs=1))
        act = ctx.enter_context(tc.tile_pool(name="act", bufs=1))
        pmm = ctx.enter_context(tc.tile_pool(name="pmm", bufs=2, space="PSUM"))
        ptr = ctx.enter_context(tc.tile_pool(name="ptr", bufs=2, space="PSUM"))
        pst = ctx.enter_context(tc.tile_pool(name="pst", bufs=1, space="PSUM"))

        id_f32 = consts.tile([128, 128], F32)
        masks.make_identity(nc, id_f32[:])
        id_bf = consts.tile([128, 128], BF16)
        masks.make_identity(nc, id_bf[:])
        ones_row = consts.tile([1, 512], BF16)
        nc.gpsimd.memset(ones_row[:], 1.0)
        ones_col = consts.tile([128, 1], BF16)
        nc.gpsimd.memset(ones_col[:], 1.0)
        eps_col = consts.tile([128, 1], F32)
        nc.gpsimd.memset(eps_col[:], EPS)

        def wload(tag, d, shape, dt=BF16):
            t = wpool.tile(list(shape), dt, tag=tag)
            nc.sync.dma_start(out=t[:], in_=d[:])
            return t

        # small always-resident params
        bal_t = wload("bal", bal_d, [128, DC], F32)
        bqkv_t = wload("bqkv", bqkv_d, [128, QC], F32)
        bprow_t = wload("bprow", bprow_d, [1, DC * 128])
        bfus_t = wload("bfus", bfus_d, [128, DC], F32)
        bal2_t = wload("bal2", bal2_d, [128, DC], F32)
        bfc1_t = wload("bfc1", bfc1_d, [128, HC], F32)
        bfc2_t = wload("bfc2", bfc2_d, [128, DC], F32)
        g1_t = wload("g1c", g1_d, [128, DC], F32)
        bt1_t = wload("bt1c", bt1_d, [128, DC])

        # big weight slots (reused across passes)
        wal_t = wload("wA", wal_d, [128, 2 * DC, DC, 128])
        wqkv_t = wload("wB", wqkv_d, [128, DC, QC, 128])
        wproj_t = wload("wC", wproj_d, [128, DC, DC, 128])
        wfus_t = wload("wD", wfus_d, [128, DC, DC, 128])

        # persistent activations
        ac = [[act.tile([128, DC, NT], BF16, tag=f"ac{m}{b}",
                        name=f"ac{m}{b}")
               for b in range(NB)] for m in MODS]
        vfc = [act.tile([128, DC, NT], BF16, tag=f"vfc{b}", name=f"vfc{b}")
               for b in range(NB)]

        # ---- helper: LN stats (a=rstd, b=-mean*rstd rows, bf16) ----
        def ln_stats(pool, src_chunks, ntok, a_dst, b_dst):
            ps_s = pst.tile([1, ntok], F32, tag="st_s")
            ps_q = pst.tile([1, ntok], F32, tag="st_q")
            for cc in range(DC):
                sq = pool.tile([128, ntok], BF16, tag="st_sq")
                nc.scalar.activation(sq[:], src_chunks[cc], AF.Square)
                nc.tensor.matmul(ps_s[:], ones_col[:], src_chunks[cc],
                                 start=(cc == 0), stop=(cc == DC - 1))
                nc.tensor.matmul(ps_q[:], ones_col[:], sq[:],
                                 start=(cc == 0), stop=(cc == DC - 1))
            m_r = pool.tile([1, ntok], F32, tag="st_m")
            nc.vector.tensor_scalar(m_r[:], ps_s[:], 1.0 / DIM, None, OP.mult)
            t1 = pool.tile([1, ntok], F32, tag="st_mq")
            nc.vector.tensor_scalar(t1[:], ps_q[:], 1.0 / DIM, None, OP.mult)
            t2 = pool.tile([1, ntok], F32, tag="st_mm")
            nc.vector.tensor_tensor(t2[:], m_r[:], m_r[:], OP.mult)
            var = pool.tile([1, ntok], F32, tag="st_var")
            nc.vector.tensor_tensor(var[:], t1[:], t2[:], OP.subtract)
            lnv = pool.tile([1, ntok], F32, tag="st_lnv")
            nc.scalar.activation(lnv[:], var[:], AF.Ln, bias=eps_col[0:1, :])
            r_r = pool.tile([1, ntok], F32, tag="st_r")
            nc.scalar.activation(r_r[:], lnv[:], AF.Exp, scale=-0.5)
            nc.vector.tensor_copy(a_dst, r_r[:])
            mr = pool.tile([1, ntok], F32, tag="st_mr")
            nc.vector.tensor_tensor(mr[:], m_r[:], r_r[:], OP.mult)
            nc.vector.tensor_scalar(b_dst, mr[:], -1.0, None, OP.mult)

        def bcast(row_ap, n, tag):
            ps = ptr.tile([128, n], F32, tag=tag)
            nc.tensor.matmul(ps[:], ones_row[:, 0:128], row_ap, start=True, stop=True)
            return ps

        # ==================== PASS 1 ====================
        with tc.tile_pool(name="s1", bufs=1) as s1, \
             tc.tile_pool(name="s1b", bufs=2) as s1b:
            for b in range(NB):
                xt = []
                ar = []
                br = []
                # ---- A: load + transpose to channel-major + stats ----
                for m in MODS:
                    xt_m = s1.tile([128, DC, NT], BF16, tag=f"xt{m}")
                    for (t0, tsz) in _tok_chunks(NT):
                        tm = s1.tile([128, DIM], F32, tag="in_tm")
                        nc.sync.dma_start(out=tm[:tsz, :],
                                          in_=xin[m][b, t0:t0 + tsz, :])
                        for cc in range(DC):
                            ps = ptr.tile([128, 128], F32, tag="tr_ps")
                            nc.tensor.transpose(
                                ps[:, :tsz],
                                tm[:tsz, cc * 128:(cc + 1) * 128],
                                id_f32[:tsz, :tsz])
                            if cc % 2 == 0:
                                nc.vector.tensor_copy(
                                    xt_m[:, cc, t0:t0 + tsz], ps[:, :tsz])
                            else:
                                nc.scalar.copy(
                                    xt_m[:, cc, t0:t0 + tsz], ps[:, :tsz])
                    a_m = s1.tile([1, NT], BF16, tag=f"ar{m}")
                    b_m = s1.tile([1, NT], BF16, tag=f"br{m}")
                    ln_stats(s1, [xt_m[:, cc, :] for cc in range(DC)], NT,
                             a_m[:], b_m[:])
                    xt.append(xt_m)
                    ar.append(a_m)
                    br.append(b_m)

                # ---- B: z_f ----
                if _STOP == "a":
                    continue
                zf = s1.tile([128, DC, ZL], BF16, tag="zf")
                for co in range(DC):
                    ps = pmm.tile([128, ZL], F32, tag="mm_ps")
                    for ci in range(2 * DC):
                        src = (xt[0][:, ci, 0:ZL] if ci < DC
                               else xt[1][:, ci - DC, 0:ZL])
                        nc.tensor.matmul(ps[:], wal_t[:, ci, co, :], src,
                                         start=(ci == 0),
                                         stop=(ci == 2 * DC - 1))
                    nc.vector.tensor_scalar(zf[:, co, :], ps[:],
                                            bal_t[:, co:co + 1], None, OP.add)
                za_r = s1.tile([1, ZL], BF16, tag="zar")
                zb_r = s1.tile([1, ZL], BF16, tag="zbr")
                ln_stats(s1, [zf[:, cc, :] for cc in range(DC)], ZL,
                         za_r[:], zb_r[:])
                nzf = s1.tile([128, DC, ZL], BF16, tag="nzf")
                zg = s1.tile([128, DC, ZL], BF16, tag="zg")
                za_bc = bcast(za_r[:], ZL, "bc")
                zb_bc = bcast(zb_r[:], ZL, "bc")
                for cc in range(DC):
                    t1 = s1b.tile([128, ZL], BF16, tag="nzf_t1")
                    nc.vector.tensor_tensor(t1[:], zf[:, cc, :], za_bc[:],
                                            OP.mult)
                    nc.vector.tensor_tensor(nzf[:, cc, :], t1[:], zb_bc[:],
                                            OP.add)
                    nc.vector.tensor_scalar(zg[:, cc, :], zf[:, cc, :],
                                            g1_t[:, cc:cc + 1], None, OP.mult)
                bz = s1.tile([1, ZL], BF16, tag="bz")
                ps_bz = pst.tile([1, ZL], F32, tag="st_s")
                for cc in range(DC):
                    nc.tensor.matmul(ps_bz[:], bt1_t[:, cc:cc + 1],
                                     zf[:, cc, :],
                                     start=(cc == 0), stop=(cc == DC - 1))
                nc.vector.tensor_copy(bz[:], ps_bz[:])

                # ---- C: qkv + attention + proj (per modality) ----
                if _STOP == "b":
                    continue
                nx = []
                for m in MODS:
                    a_bc = bcast(ar[m][:], NT, "bc")
                    b_bc = bcast(br[m][:], NT, "bc")
                    nx_m = s1.tile([128, DC, NT], BF16, tag=f"nx{m}")
                    for cc in range(DC):
                        t1 = s1b.tile([128, NT], BF16, tag="nx_t1")
                        nc.vector.tensor_tensor(t1[:], xt[m][:, cc, :],
                                                a_bc[:], OP.mult)
                        nc.vector.tensor_tensor(nx_m[:, cc, :], t1[:],
                                                b_bc[:], OP.add)
                    nx.append(nx_m)
                    qkv = s1.tile([128, QC, NC3], BF16, tag="qkv")
                    for co in range(QC):
                        ps = pmm.tile([128, NC3], F32, tag="mm_ps")
                        for ci in range(DC):
                            nc.tensor.matmul(ps[:, 0:ZL],
                                             wqkv_t[:, ci, co, :],
                                             nzf[:, ci, :],
                                             start=(ci == 0),
                                             stop=(ci == DC - 1))
                        for ci in range(DC):
                            nc.tensor.matmul(ps[:, ZL:NC3],
                                             wqkv_t[:, ci, co, :],
                                             nx_m[:, ci, :],
                                             start=(ci == 0),
                                             stop=(ci == DC - 1))
                        if co % 2 == 0:
                            nc.vector.tensor_scalar(qkv[:, co, :], ps[:],
                                                    bqkv_t[:, co:co + 1],
                                                    None, OP.add)
                        else:
                            nc.scalar.activation(qkv[:, co, :], ps[:],
                                                 AF.Identity,
                                                 bias=bqkv_t[:, co:co + 1])
                    o_t = s1.tile([128, DC, NC3], BF16, tag="o_t")
                    for h in range(HEADS):
                        p0 = (h % 2) * HD
                        cq, ck, cv = h // 2, DC + h // 2, 2 * DC + h // 2
                        e_t = s1b.tile([128, 3, NC3], BF16, tag="e_t")
                        zcol = s1b.tile([128, 3], F32, tag="zcol")
                        for qc in range(3):
                            ps_s = pmm.tile([128, NC3], F32, tag="mm_ps")
                            nc.tensor.matmul(
                                ps_s[:],
                                qkv[p0:p0 + HD, cq,
                                    qc * 128:(qc + 1) * 128],
                                qkv[p0:p0 + HD, ck, :],
                                start=True, stop=True)
                            nc.scalar.activation(
                                e_t[:, qc, :], ps_s[:], AF.Exp,
                                scale=1.0 / 8.0,
                                accum_out=zcol[:, qc:qc + 1])
                        rinv = s1b.tile([128, 3], F32, tag="rinv")
                        nc.vector.reciprocal(rinv[:], zcol[:])
                        p_t = e_t
                        for qc in range(3):
                            nc.vector.tensor_scalar(p_t[:, qc, :],
                                                    e_t[:, qc, :],
                                                    rinv[:, qc:qc + 1],
                                                    None, OP.mult)
                            nc.sync.dma_start(
                                out=attn_out[m][b, h,
                                                qc * 128:(qc + 1) * 128, :],
                                in_=p_t[:, qc, :])
                        pt_t = s1b.tile([128, 3, NC3], BF16, tag="pt_t")
                        for kc in range(3):
                            for qc in range(3):
                                ps_t = ptr.tile([128, 128], BF16, tag="tr_ps")
                                nc.tensor.transpose(
                                    ps_t[:],
                                    p_t[:, qc, kc * 128:(kc + 1) * 128],
                                    id_bf[:])
                                if (kc + qc) % 2 == 0:
                                    nc.vector.tensor_copy(
                                        pt_t[:, kc,
                                             qc * 128:(qc + 1) * 128],
                                        ps_t[:])
                                else:
                                    nc.scalar.copy(
                                        pt_t[:, kc,
                                             qc * 128:(qc + 1) * 128],
                                        ps_t[:])
                        vt_t = s1b.tile([128, 3, HD], BF16, tag="vt_t")
                        for kc in range(3):
                            ps_t = ptr.tile([128, 128], BF16, tag="tr_ps")
                            nc.tensor.transpose(
                                ps_t[:, 0:HD],
                                qkv[p0:p0 + HD, cv,
                                    kc * 128:(kc + 1) * 128],
                                id_bf[p0:p0 + HD, p0:p0 + HD])
                            nc.scalar.copy(vt_t[:, kc, :], ps_t[:, 0:HD])
                        ps_o = pmm.tile([HD, NC3], F32, tag="mm_ps")
                        for kc in range(3):
                            nc.tensor.matmul(ps_o[:], vt_t[:, kc, :],
                                             pt_t[:, kc, :],
                                             start=(kc == 0), stop=(kc == 2))
                        if h % 2 == 0:
                            nc.vector.tensor_copy(o_t[p0:p0 + HD, h // 2, :],
                                                  ps_o[:])
                        else:
                            nc.scalar.copy(o_t[p0:p0 + HD, h // 2, :],
                                           ps_o[:])
                    # proj; ac = x_ori + attn_c (+bias via K=1 row matmul)
                    for co in range(DC):
                        ps = pmm.tile([128, NC3], F32, tag="mm_ps")
                        for ci in range(DC):
                            nc.tensor.matmul(ps[:], wproj_t[:, ci, co, :],
                                             o_t[:, ci, :],
                                             start=(ci == 0), stop=False)
                        nc.tensor.matmul(ps[:], bprow_t[0:1, co * 128:(co + 1) * 128],
                                         ones_row[:, 0:NC3],
                                         start=False, stop=True)
                        nc.vector.tensor_tensor(ac[m][b][:, co, :],
                                                ps[:, ZL:NC3],
                                                xt[m][:, co, :], OP.add)
                        if m == 1:
                            nc.scalar.copy(vfc[b][:, co, 0:ZL], ps[:, 0:ZL])

                # ---- D: attn_x fusion + entropy ----
                if _STOP == "c":
                    continue
                ucols = []
                for m in MODS:
                    zax = s1b.tile([128, 2], F32, tag=f"zax{m}")
                    s2c = s1b.tile([128, 2], F32, tag=f"s2{m}")
                    for sc in range(2):
                        ps_ax = pmm.tile([128, ZL], F32, tag="mm_ps")
                        for ci in range(DC):
                            nc.tensor.matmul(
                                ps_ax[:],
                                nx[m][:, ci,
                                      ZL + sc * 128:ZL + (sc + 1) * 128],
                                zg[:, ci, :],
                                start=(ci == 0), stop=False)
                        nc.tensor.matmul(ps_ax[:], ones_row[:, 0:128], bz[:],
                                         start=False, stop=True)
                        ex = s1.tile([128, ZL], F32, tag="ex")
                        nc.scalar.activation(ex[:], ps_ax[:], AF.Exp,
                                             scale=SCALE,
                                             accum_out=zax[:, sc:sc + 1])
                        prod = s1.tile([128, ZL], F32, tag="ttr_dump")
                        nc.vector.tensor_tensor(prod[:], ex[:], ps_ax[:],
                                                OP.mult)
                        nc.vector.tensor_reduce(
                            s2c[:, sc:sc + 1], prod[:],
                            mybir.AxisListType.X, OP.add)
                    zinv = s1b.tile([128, 2], F32, tag=f"zinv{m}")
                    nc.vector.reciprocal(zinv[:], zax[:])
                    t1 = s1b.tile([128, 2], F32, tag=f"ut1{m}")
                    nc.vector.tensor_tensor(t1[:], s2c[:], zinv[:], OP.mult)
                    t2 = s1b.tile([128, 2], F32, tag=f"ut2{m}")
                    nc.vector.tensor_scalar(t2[:], t1[:], -SCALE, None,
                                            OP.mult)
                    lnz = s1b.tile([128, 2], F32, tag=f"lnz{m}")
                    nc.scalar.activation(lnz[:], zax[:], AF.Ln)
                    uu = s1b.tile([128, 2], F32, tag=f"uu{m}")
                    nc.vector.tensor_tensor(uu[:], lnz[:], t2[:], OP.add)
                    ucols.append(uu)
                    for sc in range(2):
                        nc.sync.dma_start(
                            out=out_u[m][b, sc * 128:(sc + 1) * 128],
                            in_=uu[:, sc:sc + 1])
                u_x, u_xi = ucols
                um_t = s1b.tile([128, 2], F32, tag="um")
                nc.vector.tensor_tensor(um_t[:], u_x[:], u_xi[:], OP.add)
                um2 = s1b.tile([128, 2], F32, tag="um2")
                nc.vector.tensor_scalar(um2[:], um_t[:], 0.5, None, OP.mult)
                du = s1b.tile([128, 2], F32, tag="du")
                nc.vector.tensor_tensor(du[:], u_x[:], u_xi[:], OP.subtract)
                edu = s1b.tile([128, 2], F32, tag="edu")
                nc.scalar.activation(edu[:], du[:], AF.Exp)
                ep1 = s1b.tile([128, 2], F32, tag="ep1")
                nc.vector.tensor_scalar(ep1[:], edu[:], 1.0, None, OP.add)
                alpha = s1b.tile([128, 2], F32, tag="alpha")
                nc.vector.reciprocal(alpha[:], ep1[:])
                for sc in range(2):
                    nc.sync.dma_start(out=um_d[b, sc * 128:(sc + 1) * 128],
                                      in_=um2[:, sc:sc + 1])
                    nc.sync.dma_start(out=al_d[b, sc * 128:(sc + 1) * 128],
                                      in_=alpha[:, sc:sc + 1])
                # alpha broadcast row
                alpha_bf = s1b.tile([128, 2], BF16, tag="alphabf")
                nc.vector.tensor_copy(alpha_bf[:], alpha[:])
                arow_t = s1.tile([1, SL], BF16, tag="arow_t")
                if "atr" not in _SKIP:
                    for sc in range(2):
                        ps_at = ptr.tile([128, 128], BF16, tag="tr_ps")
                        nc.tensor.transpose(ps_at[0:1, :],
                                            alpha_bf[:, sc:sc + 1], id_bf[:])
                        nc.vector.tensor_copy(
                            arow_t[:, sc * 128:(sc + 1) * 128], ps_at[0:1, :])
                else:
                    nc.gpsimd.memset(arow_t[:], 0.5)
                ab_ps = bcast(arow_t[:], SL, "bc")
                ab_t = s1.tile([128, SL], BF16, tag="ab_t")
                nc.scalar.copy(ab_t[:], ab_ps[:])
                # v_f = B + alpha*(A-B)
                for co in range(DC if "vf" not in _SKIP else 0):
                    ps_a = pmm.tile([128, SL], F32, tag="mm_ps")
                    ps_b = pmm.tile([128, SL], F32, tag="mm_ps")
                    for ci in range(DC):
                        nc.tensor.matmul(ps_a[:], wfus_t[:, ci, co, :],
                                         nx[0][:, ci, ZL:NT],
                                         start=(ci == 0), stop=(ci == DC - 1))
                    for ci in range(DC):
                        nc.tensor.matmul(ps_b[:], wfus_t[:, ci, co, :],
                                         nx[1][:, ci, ZL:NT],
                                         start=(ci == 0), stop=(ci == DC - 1))
                    a_sb = s1.tile([128, SL], BF16, tag="vfA")
                    nc.scalar.activation(a_sb[:], ps_a[:], AF.Identity,
                                         bias=bfus_t[:, co:co + 1])
                    b_sb = s1.tile([128, SL], BF16, tag="vfB")
                    nc.scalar.activation(b_sb[:], ps_b[:], AF.Identity,
                                         bias=bfus_t[:, co:co + 1])
                    d_sb = s1.tile([128, SL], BF16, tag="vfd")
                    nc.vector.tensor_tensor(d_sb[:], a_sb[:], b_sb[:],
                                            OP.subtract)
                    m_sb = s1.tile([128, SL], BF16, tag="vfm")
                    nc.vector.tensor_tensor(m_sb[:], d_sb[:], ab_t[:],
                                            OP.mult)
                    nc.vector.tensor_tensor(vfc[b][:, co, ZL:NT], m_sb[:],
                                            b_sb[:], OP.add)

        # ==================== PASS 2 ====================
        wfc1_t = wload("wA", wfc1_d, [128, DC, HC, 128]) \
            if _STOP == "full" else None
        wfc2_t = wload("wB", wfc2_d, [128, HC, DC, 128]) \
            if _STOP == "full" else None
        wal2_t = wload("wC", wal2_d, [128, DC, DC, 128]) \
            if _STOP == "full" else None
        with tc.tile_pool(name="s2", bufs=1) as s2, \
             tc.tile_pool(name="s2b", bufs=2) as s2b:
            for b in range(NB if _STOP == "full" else 0):
                # ---- E: v_fc stats + adap2 (ad includes bfc2) ----
                va_r = s2.tile([1, NT], BF16, tag="va_r")
                vb_r = s2.tile([1, NT], BF16, tag="vb_r")
                ln_stats(s2, [vfc[b][:, cc, :] for cc in range(DC)], NT,
                         va_r[:], vb_r[:])
                va_bc = bcast(va_r[:], NT, "bc")
                vb_bc = bcast(vb_r[:], NT, "bc")
                nv = s2.tile([128, DC, NT], BF16, tag="nv")
                for cc in range(DC):
                    t1 = s2b.tile([128, NT], BF16, tag="nv_t1")
                    nc.vector.tensor_tensor(t1[:], vfc[b][:, cc, :],
                                            va_bc[:], OP.mult)
                    nc.vector.tensor_tensor(nv[:, cc, :], t1[:], vb_bc[:],
                                            OP.add)
                ad = s2b.tile([128, DC, NT], BF16, tag="ad")
                for co in range(DC):
                    ps = pmm.tile([128, NT], F32, tag="mm_ps")
                    for ci in range(DC):
                        nc.tensor.matmul(ps[:], wal2_t[:, ci, co, :],
                                         nv[:, ci, :],
                                         start=(ci == 0), stop=(ci == DC - 1))
                    nc.vector.tensor_scalar(ad[:, co, :], ps[:],
                                            bal2_t[:, co:co + 1],
                                            bfc2_t[:, co:co + 1],
                                            OP.add, OP.add)
                # ---- F/G: x_mid, MLP, final ----
                for m in MODS:
                    xmid = s2b.tile([128, DC, NT], BF16, tag="xmid")
                    for cc in range(DC):
                        nc.vector.tensor_tensor(xmid[:, cc, :],
                                                ac[m][b][:, cc, :],
                                                vfc[b][:, cc, :], OP.add)
                    ma_r = s2.tile([1, NT], BF16, tag="ma_r")
                    mb_r = s2.tile([1, NT], BF16, tag="mb_r")
                    ln_stats(s2, [xmid[:, cc, :] for cc in range(DC)], NT,
                             ma_r[:], mb_r[:])
                    ma_bc = bcast(ma_r[:], NT, "bc")
                    mb_bc = bcast(mb_r[:], NT, "bc")
                    nxm = s2.tile([128, DC, NT], BF16, tag="nxm")
                    for cc in range(DC):
                        t1 = s2b.tile([128, NT], BF16, tag="nxm_t1")
                        nc.vector.tensor_tensor(t1[:], xmid[:, cc, :],
                                                ma_bc[:], OP.mult)
                        nc.vector.tensor_tensor(nxm[:, cc, :], t1[:],
                                                mb_bc[:], OP.add)
                    h_t = s2.tile([128, HC, NT], BF16, tag="h_t")
                    for ho in range(HC):
                        ps = pmm.tile([128, NT], F32, tag="mm_ps")
                        for ci in range(DC):
                            nc.tensor.matmul(ps[:], wfc1_t[:, ci, ho, :],
                                             nxm[:, ci, :],
                                             start=(ci == 0),
                                             stop=(ci == DC - 1))
                        nc.scalar.activation(h_t[:, ho, :], ps[:], AF.Gelu,
                                             bias=bfc1_t[:, ho:ho + 1])
                    xout = s2.tile([128, DC, NT], F32, tag="xout")
                    for co in range(DC):
                        ps = pmm.tile([128, NT], F32, tag="mm_ps")
                        for hc in range(HC):
                            nc.tensor.matmul(ps[:], wfc2_t[:, hc, co, :],
                                             h_t[:, hc, :],
                                             start=(hc == 0),
                                             stop=(hc == HC - 1))
                        t1 = s2b.tile([128, NT], F32, tag="fin_t1")
                        nc.vector.tensor_tensor(t1[:], ps[:],
                                                ad[:, co, :], OP.add)
                        nc.vector.tensor_tensor(xout[:, co, :], t1[:],
                                                xmid[:, co, :], OP.add)
                    for (t0, tsz) in _tok_chunks(NT):
                        otm = s2b.tile([128, DIM], F32, tag="otm")
                        for cc in range(DC):
                            ps_t = ptr.tile([128, 128], F32, tag="tr_ps")
                            nc.tensor.transpose(ps_t[:tsz, :],
                                                xout[:, cc, t0:t0 + tsz],
                                                id_f32[:, :])
                            if cc % 2 == 0:
                                nc.vector.tensor_copy(
                                    otm[:tsz, cc * 128:(cc + 1) * 128],
                                    ps_t[:tsz, :])
                            else:
                                nc.scalar.copy(
                                    otm[:tsz, cc * 128:(cc + 1) * 128],
                                    ps_t[:tsz, :])
                        nc.sync.dma_start(out=xout_d[m][b, t0:t0 + tsz, :],
                                          in_=otm[:tsz, :])

    nc.compile()
    return nc


# =====================================================================
# Host-side weight prep
# =====================================================================
def _chunk_w(w_eff):
    """[O, C] -> [128, C/128, O/128, 128]: w_t[p, ci, co, f] =
    W[co*128+f, ci*128+p] (lhsT chunks)."""
    O, C = w_eff.shape
    a = w_eff.reshape(O // 128, 128, C // 128, 128)  # [co, f, ci, p]
    return np.ascontiguousarray(a.transpose(3, 2, 0, 1)).astype(
        ml_dtypes.bfloat16)


def _chunk_b(b_eff, dt=ml_dtypes.bfloat16):
    return np.ascontiguousarray(b_eff.reshape(-1, 128).T).astype(dt)


def _prep_weights(inp):
    f32 = np.float32
    g1 = np.asarray(inp["g1"], f32)
    bt1 = np.asarray(inp["bt1"], f32)
    g2 = np.asarray(inp["g2"], f32)
    bt2 = np.asarray(inp["bt2"], f32)
    out = {}
    w_al = np.asarray(inp["w_al"], f32)
    out["wal"] = _chunk_w(w_al)
    out["bal"] = _chunk_b(np.asarray(inp["b_al"], f32), f32)
    w_qkv = np.asarray(inp["w_qkv"], f32)
    out["wqkv"] = _chunk_w(w_qkv * g1[None, :])
    out["bqkv"] = _chunk_b(np.asarray(inp["b_qkv"], f32) + w_qkv @ bt1, f32)
    out["wproj"] = _chunk_w(np.asarray(inp["w_proj"], f32))
    out["bprow"] = np.ascontiguousarray(
        np.asarray(inp["b_proj"], f32).reshape(1, -1)).astype(
            ml_dtypes.bfloat16)
    w_fus = np.asarray(inp["w_fus"], f32)
    out["wfus"] = _chunk_w(w_fus * g1[None, :])
    out["bfus"] = _chunk_b(np.asarray(inp["b_fus"], f32) + w_fus @ bt1, f32)
    w_al2 = np.asarray(inp["w_al2"], f32)
    out["wal2"] = _chunk_w(w_al2 * g2[None, :])
    out["bal2"] = _chunk_b(np.asarray(inp["b_al2"], f32) + w_al2 @ bt2, f32)
    w_fc1 = np.asarray(inp["w_fc1"], f32)
    out["wfc1"] = _chunk_w(w_fc1 * g2[None, :])
    out["bfc1"] = _chunk_b(np.asarray(inp["b_fc1"], f32) + w_fc1 @ bt2, f32)
    out["wfc2"] = _chunk_w(np.asarray(inp["w_fc2"], f32))
    out["bfc2"] = _chunk_b(np.asarray(inp["b_fc2"], f32), f32)
    out["g1c"] = _chunk_b(g1, f32)
    out["bt1c"] = _chunk_b(bt1)
    return out


def _get_nc():
    if "nc" not in _CACHED:
        _CACHED["nc"] = build_nc()
    return _CACHED["nc"]


def kernel(**inputs):
    nc = _get_nc()
    w = _prep_weights(inputs)
    x = np.asarray(inputs["x"], dtype=np.float32)
    xi = np.asarray(inputs["xi"], dtype=np.float32)
    in_maps = []
    for c in range(NCORES):
        m = dict(w)
        m["x"] = np.ascontiguousarray(x[c * NB:(c + 1) * NB])
        m["xi"] = np.ascontiguousarray(xi[c * NB:(c + 1) * NB])
        in_maps.append(m)
    res = run_bass_kernel_spmd(nc, in_maps, core_ids=list(range(NCORES)))
    r = res.results

    def gather(name):
        return np.concatenate([np.asarray(r[c][name]).astype(np.float32)
                               for c in range(NCORES)], axis=0)

    return (gather("xo"), inputs["global_index_template"],
            inputs["global_index_search"], gather("attn"),
            gather("xio"), inputs["global_index_templatei"],
            inputs["global_index_searchi"], gather("attni"),
            gather("alpha"), gather("um"), gather("u"), gather("ui"))
